# revision 2
# baseline (speedup 1.0000x reference)
"""BinaryLinear Trainium2 kernel: out = x @ sign(weight).T + bias.

x [8192, 4096] f32, weight [4096, 4096] f32, bias [4096] f32.
Data-parallel over tokens across 8 NeuronCores (1024 tokens/core,
weight+bias replicated, no collectives).

Per-core pipeline (see git history for the measured evolution):
  1. W/x rows are loaded as PLAIN f32 on the scalar HWDGE ring - NOT as
     SWDGE cast-DMAs. Crucial: SWDGE (gpsimd) descriptor generation is
     locked out of the shared SBUF port pair whenever DVE runs a 2-port
     perf-mode op (the f32 psum drains), so cast-DMAs serialize against
     the whole DVE stream (~+450us). HWDGE is immune.
  2. ACT fuses sign+cast in one pass: W f32 -> fp8e4 (+-1 exact; the
     1e30 scale pushes tiny values off the LUT's zero breakpoint while
     keeping sign(0)=0), written pair-interleaved: fp8 position
     256*cb + 2*p + b holds sign(W[o, 256*cb + 128*b + p]).
     x: f32 -> bf16 via Copy activation.
  3. W transposes (sync-ring XBAR) run on the uint16 VIEW of the fp8
     pair array - HALF the bytes of a bf16 transpose. The transposed
     pair tile t2[p, cb, o] then holds signs for k=2cb+b at i=128k+p.
  4. DVE unpacks fp8->bf16 into wt_h[p, cb, b, o] so matmul rhs slices
     wt_h[:, (k%16)>>1, k&1, :] are plain contiguous [128, 512] views
     with wt[p,k,o] = sign(W[o, 128k+p]) - identical layout/semantics
     to a direct bf16 transpose.
  5. TensorE: psum[m] += xt[k,m].T @ wt[k], fp32 PSUM accumulation over
     32 k-tiles, two phase-shifted groups of 4 PSUM banks.
  6. DVE adds the (head-preloaded) bias during PSUM->SBUF copy; stores
     go out on the scalar HWDGE ring.
"""

import numpy as np

import concourse.mybir as mybir
import concourse.tile as tile
from concourse import bacc
from concourse.bass import ts

P = 128
TOKENS, IN_F, OUT_F = 8192, 4096, 4096
N_CORES = 8
N_TILE = 512

F32 = mybir.dt.float32
BF16 = mybir.dt.bfloat16
FP8 = mybir.dt.float8e4
U16 = mybir.dt.uint16


def build_nc(t_shard=TOKENS // N_CORES, in_f=IN_F, out_f=OUT_F, repeat=1):
    m_tiles = t_shard // P
    n_tiles = out_f // N_TILE
    ko_tiles = in_f // P          # 32
    j_tiles = N_TILE // P         # 4
    cb_tiles = ko_tiles // 2      # 16
    half_k = ko_tiles // 2        # 16
    cb_half = cb_tiles // 2       # 8

    nc = bacc.Bacc(None, target_bir_lowering=False, debug=False)

    x = nc.dram_tensor("x", [t_shard, in_f], F32, kind="ExternalInput")
    w = nc.dram_tensor("weight", [out_f, in_f], F32, kind="ExternalInput")
    b = nc.dram_tensor("bias", [out_f], F32, kind="ExternalInput")
    out = nc.dram_tensor("out", [t_shard, out_f], F32, kind="ExternalOutput")

    with tile.TileContext(nc) as tc:
        with (
            tc.tile_pool(name="consts", bufs=8) as const_pool,
            tc.tile_pool(name="stage", bufs=3) as stage_pool,
            tc.tile_pool(name="xb", bufs=2) as xb_pool,
            tc.tile_pool(name="pk", bufs=2) as pk_pool,
            tc.tile_pool(name="xt", bufs=1) as xt_pool,
            tc.tile_pool(name="wt", bufs=2) as wt_pool,
            tc.tile_pool(name="out_sb", bufs=6) as out_pool,
            tc.tile_pool(name="ps", bufs=8, space="PSUM") as psum_pool,
        ):
          for _rep in range(repeat):
            biases = []
            for n in range(n_tiles):
                bias_rep = const_pool.tile(
                    [P, N_TILE], F32, name=f"bias_rep_{n}", tag="bias"
                )
                nc.gpsimd.dma_start(
                    bias_rep,
                    b[None, ts(n, N_TILE)].broadcast_to([P, N_TILE]),
                )
                biases.append(bias_rep)

            def load_slab_f32(src_rows):
                slab = stage_pool.tile([P, in_f], F32, name="slab", tag="stage")
                nc.scalar.dma_start(slab, src_rows)
                return slab

            def emit_wt(n):
                t2s = []
                for j in range(j_tiles):
                    slab = load_slab_f32(w[ts(n * j_tiles + j, P), :])
                    s8 = pk_pool.tile(
                        [P, cb_tiles, P, 2], FP8, name=f"s8_{n}_{j}", tag="s8"
                    )
                    slab4 = slab[:, :].rearrange(
                        "o (cb bb p) -> o cb bb p", cb=cb_tiles, bb=2, p=P
                    )
                    for bb in range(2):
                        nc.scalar.activation(
                            s8[:, :, :, bb], slab4[:, :, bb, :],
                            mybir.ActivationFunctionType.Sign, scale=1.0e30,
                        )
                    t2 = pk_pool.tile(
                        [P, cb_tiles, P], U16, name=f"t2_{n}_{j}", tag="t2"
                    )
                    # NOTE: XBAR transposes must stay on nc.sync (scalar-ring
                    # transposes corrupt on HW)
                    nc.sync.dma_start(
                        t2, s8[:, :, :, :].bitcast(U16), transpose=True
                    )
                    t2s.append(t2)
                halves = []
                for h in range(2):
                    wt_h = wt_pool.tile(
                        [P, cb_half, 2, N_TILE], BF16,
                        name=f"wt_{n}_{h}", tag="wt"
                    )
                    for j, t2 in enumerate(t2s):
                        src = t2[:, ts(h, cb_half), :].bitcast(FP8).rearrange(
                            "p cb (o bb) -> p cb bb o", o=P, bb=2
                        )
                        nc.vector.tensor_copy(
                            wt_h[:, :, :, ts(j, P)], src
                        )
                    halves.append(wt_h)
                return halves

            xt_all = xt_pool.tile(
                [P, ko_tiles, t_shard], BF16, name="xt_all", tag="xt"
            )
            for m in range(m_tiles):
                slab = load_slab_f32(x[ts(m, P), :])
                xb = xb_pool.tile([P, in_f], BF16, name="xb", tag="xb")
                nc.scalar.activation(
                    xb, slab, mybir.ActivationFunctionType.Copy
                )
                nc.sync.dma_start(
                    xt_all[:, :, ts(m, P)], xb, transpose=True
                )
            wts = {0: emit_wt(0)}

            for n in range(n_tiles):
                if n + 1 < n_tiles:
                    wts[n + 1] = emit_wt(n + 1)
                wt_n = wts.pop(n)

                half = max(1, m_tiles // 2)
                for g0 in range(0, m_tiles, half):
                    ms = range(g0, min(g0 + half, m_tiles))
                    psums = {
                        m: psum_pool.tile(
                            [P, N_TILE], F32, name=f"ps_{n}_{m}", tag="ps"
                        )
                        for m in ms
                    }
                    for k in range(ko_tiles):
                        wt_h = wt_n[k // half_k]
                        rhs = wt_h[:, (k % half_k) >> 1, k & 1, :]
                        for m in ms:
                            nc.tensor.matmul(
                                psums[m],
                                xt_all[:, k, ts(m, P)],
                                rhs,
                                start=(k == 0),
                                stop=(k == ko_tiles - 1),
                            )
                    for m in ms:
                        out_sb = out_pool.tile(
                            [P, N_TILE], F32, name="out_sb", tag="out_sb"
                        )
                        nc.vector.tensor_tensor(
                            out_sb, psums[m], biases[n], mybir.AluOpType.add
                        )
                        nc.scalar.dma_start(
                            out[ts(m, P), ts(n, N_TILE)], out_sb
                        )

    nc.compile()
    return nc


_NC_CACHE = {}


def _get_nc(shape_key):
    if shape_key not in _NC_CACHE:
        _NC_CACHE[shape_key] = build_nc(*shape_key)
    return _NC_CACHE[shape_key]


def _run(nc, x, weight, bias, trace):
    from concourse.bass_utils import run_bass_kernel_spmd

    tokens = x.shape[0]
    t_shard = tokens // N_CORES
    in_maps = [
        {
            "x": x[c * t_shard : (c + 1) * t_shard],
            "weight": weight,
            "bias": bias,
        }
        for c in range(N_CORES)
    ]
    res = run_bass_kernel_spmd(
        nc, in_maps, core_ids=list(range(N_CORES)), trace=trace
    )
    return np.concatenate([r["out"] for r in res.results], axis=0), res


def _spot_check(out, x, weight, bias):
    """Verify 2 sampled output columns against numpy; guards against the
    rare transient where one core's execution returns zeros."""
    cols = [137, 3972]
    s = np.sign(weight[cols, :].astype(np.float32)).T
    ref = x.astype(np.float32) @ s + bias[cols][None, :]
    got = out[:, cols]
    denom = np.linalg.norm(ref)
    rel = np.linalg.norm(got - ref) / max(denom, 1e-30)
    return rel < 1e-2


def kernel(x, weight, bias, _trace=False):
    x = np.ascontiguousarray(np.asarray(x, dtype=np.float32))
    weight = np.ascontiguousarray(np.asarray(weight, dtype=np.float32))
    bias = np.ascontiguousarray(np.asarray(bias, dtype=np.float32))

    tokens = x.shape[0]
    t_shard = tokens // N_CORES
    nc = _get_nc((t_shard, x.shape[1], weight.shape[0]))

    out, res = _run(nc, x, weight, bias, _trace)
    if not _spot_check(out, x, weight, bias):
        # transient device-side failure (observed rarely as one core
        # returning zeros on a first exec) - run once more
        out, res = _run(nc, x, weight, bias, _trace)
    if _trace:
        return out, res
    return out


# revision 3
# speedup vs baseline: 1.1115x; 1.1115x over previous
"""BinaryLinear Trainium2 kernel.

Computes out = x @ sign(weight).T + bias for x [8192, 4096] f32,
weight [4096, 4096] f32, bias [4096] f32.

Strategy: data-parallel over the token dim across 8 NeuronCores
(1024 tokens per core, weight/bias replicated, no collectives).

Per-core pipeline (no DRAM scratch):
  1. x tiles [128t, 4096i] are cast f32->bf16 during the SWDGE DMA load,
     then one whole-tile XBAR transpose SBUF->SBUF lands each in
     XT [128i, 32k, 1024t] (8 transposes for x, issued first - they gate
     the first matmuls).
  2. weight rows likewise: cast to bf16 SBUF slabs [128o, 4096i]; one
     XBAR transpose per slab half fills WT_n [128i, 16k, 512o]; sign()
     is applied in place on the Scalar engine (scale=1e30 pushes tiny
     values off the LUT's zero neighborhood; sign(0)=0 preserved).
     NOTE: transposes must stay on nc.sync - the scalar HWDGE ring
     corrupts XBAR transposes on HW (passes CoreSim).
  3. TensorE: psum[m] += XT[k,m].T @ WT[n,k], fp32 accumulation in PSUM
     over all 32 k-tiles; two phase-shifted groups of 4 PSUM banks.
  4. DVE adds the (preloaded, partition-broadcast) bias while copying
     PSUM->SBUF; the scalar HWDGE ring stores f32 output tiles.

Scheduling notes (the 3.3x over the first working version):
  - Output stores live on the scalar HWDGE ring, NOT the gpsimd SWDGE
    FIFO: a store queued behind multi-us W cast DMAs stalls DVE ->
    PSUM-bank recycling -> TensorE.
  - All 8 per-block bias tiles are broadcast once at the head.
  - The PE then issues back-to-back ~102ns matmuls (N=512 bf16), which
    is the measured TensorE roofline here.
"""

import numpy as np

import concourse.mybir as mybir
import concourse.tile as tile
from concourse import bacc
from concourse.bass import ts

P = 128
TOKENS, IN_F, OUT_F = 8192, 4096, 4096
N_CORES = 8
N_TILE = 512   # output-feature block (one PSUM bank of f32)

F32 = mybir.dt.float32
BF16 = mybir.dt.bfloat16


def build_nc(t_shard=TOKENS // N_CORES, in_f=IN_F, out_f=OUT_F, repeat=1):
    m_tiles = t_shard // P      # token tiles of 128
    n_tiles = out_f // N_TILE   # output blocks of 512
    ko_tiles = in_f // P        # k tiles of 128
    j_tiles = N_TILE // P       # 128-row slabs per output block

    nc = bacc.Bacc(None, target_bir_lowering=False, debug=False)

    x = nc.dram_tensor("x", [t_shard, in_f], F32, kind="ExternalInput")
    w = nc.dram_tensor("weight", [out_f, in_f], F32, kind="ExternalInput")
    b = nc.dram_tensor("bias", [out_f], F32, kind="ExternalInput")
    out = nc.dram_tensor("out", [t_shard, out_f], F32, kind="ExternalOutput")

    with tile.TileContext(nc) as tc:
        with (
            tc.tile_pool(name="consts", bufs=8) as const_pool,
            tc.tile_pool(name="stage", bufs=6) as stage_pool,
            tc.tile_pool(name="xt", bufs=1) as xt_pool,
            tc.tile_pool(name="wt", bufs=4) as wt_pool,
            tc.tile_pool(name="out_sb", bufs=6) as out_pool,
            tc.tile_pool(name="ps", bufs=8, space="PSUM") as psum_pool,
        ):
          for _rep in range(repeat):
            biases = []
            for n in range(n_tiles):
                bias_rep = const_pool.tile(
                    [P, N_TILE], F32, name=f"bias_rep_{n}", tag="bias"
                )
                nc.gpsimd.dma_start(
                    bias_rep,
                    b[None, ts(n, N_TILE)].broadcast_to([P, N_TILE]),
                )
                biases.append(bias_rep)

            def cast_slab(src_rows):
                """SWDGE cast f32->bf16 of 128 DRAM rows into SBUF."""
                slab = stage_pool.tile([P, in_f], BF16, name="slab", tag="stage")
                nc.gpsimd.dma_start(slab, src_rows)
                return slab

            half_k = max(1, ko_tiles // 2)

            def emit_wt(n):
                """Build signed WT half-tiles [128i, 16k, 512o] for block n."""
                slabs = [
                    cast_slab(w[ts(n * j_tiles + j, P), :])
                    for j in range(j_tiles)
                ]
                halves = []
                for h in range(ko_tiles // half_k):
                    wt_h = wt_pool.tile(
                        [P, half_k, N_TILE], BF16, name=f"wt_{n}_{h}", tag="wt"
                    )
                    for j in range(j_tiles):
                        # NOTE: must stay on nc.sync (see module docstring)
                        nc.sync.dma_start(
                            wt_h[:, :, ts(j, P)],
                            slabs[j][:, ts(h, half_k * P)],
                            transpose=True,
                        )
                    # sign in place; scale pushes tiny magnitudes off the
                    # LUT's zero breakpoint while keeping sign(0) == 0
                    nc.scalar.activation(
                        wt_h, wt_h, mybir.ActivationFunctionType.Sign,
                        scale=1.0e30,
                    )
                    halves.append(wt_h)
                return halves

            # ---- head: x tiles first (each transposed whole so matmuls
            # can start after the first), then W blocks 0 and 1.
            xt_all = xt_pool.tile(
                [P, ko_tiles, t_shard], BF16, name="xt_all", tag="xt"
            )
            for m in range(m_tiles):
                slab = cast_slab(x[ts(m, P), :])
                nc.sync.dma_start(
                    xt_all[:, :, ts(m, P)], slab, transpose=True
                )
            wts = {0: emit_wt(0)}
            if n_tiles > 1:
                wts[1] = emit_wt(1)

            # ---- main loop over output blocks
            for n in range(n_tiles):
                if n + 2 < n_tiles:
                    wts[n + 2] = emit_wt(n + 2)
                wt_n = wts.pop(n)

                # two phase-shifted groups of 4 PSUM banks: group B's
                # matmuls overlap group A's output copies
                half = max(1, m_tiles // 2)
                for g0 in range(0, m_tiles, half):
                    ms = range(g0, min(g0 + half, m_tiles))
                    psums = {
                        m: psum_pool.tile(
                            [P, N_TILE], F32, name=f"ps_{n}_{m}", tag="ps"
                        )
                        for m in ms
                    }
                    for k in range(ko_tiles):
                        for m in ms:
                            nc.tensor.matmul(
                                psums[m],
                                xt_all[:, k, ts(m, P)],
                                wt_n[k // half_k][:, k % half_k, :],
                                start=(k == 0),
                                stop=(k == ko_tiles - 1),
                            )
                    for m in ms:
                        out_sb = out_pool.tile(
                            [P, N_TILE], F32, name="out_sb", tag="out_sb"
                        )
                        nc.vector.tensor_tensor(
                            out_sb, psums[m], biases[n], mybir.AluOpType.add
                        )
                        nc.scalar.dma_start(
                            out[ts(m, P), ts(n, N_TILE)], out_sb
                        )

    nc.compile()
    return nc


_NC_CACHE = {}


def _get_nc(shape_key):
    if shape_key not in _NC_CACHE:
        _NC_CACHE[shape_key] = build_nc(*shape_key)
    return _NC_CACHE[shape_key]


def _run(nc, x, weight, bias, trace):
    from concourse.bass_utils import run_bass_kernel_spmd

    tokens = x.shape[0]
    t_shard = tokens // N_CORES
    in_maps = [
        {
            "x": x[c * t_shard : (c + 1) * t_shard],
            "weight": weight,
            "bias": bias,
        }
        for c in range(N_CORES)
    ]
    res = run_bass_kernel_spmd(
        nc, in_maps, core_ids=list(range(N_CORES)), trace=trace
    )
    return np.concatenate([r["out"] for r in res.results], axis=0), res


def _spot_check(out, x, weight, bias):
    """Verify 2 sampled output columns against numpy; guards against the
    rare transient where one core's execution returns zeros/garbage."""
    cols = [137, 3972]
    s = np.sign(weight[cols, :].astype(np.float32)).T  # [in_f, 2]
    ref = x.astype(np.float32) @ s + bias[cols][None, :]
    got = out[:, cols]
    denom = np.linalg.norm(ref)
    rel = np.linalg.norm(got - ref) / max(denom, 1e-30)
    return rel < 1e-2


def kernel(x, weight, bias, _trace=False):
    x = np.ascontiguousarray(np.asarray(x, dtype=np.float32))
    weight = np.ascontiguousarray(np.asarray(weight, dtype=np.float32))
    bias = np.ascontiguousarray(np.asarray(bias, dtype=np.float32))

    tokens = x.shape[0]
    t_shard = tokens // N_CORES
    nc = _get_nc((t_shard, x.shape[1], weight.shape[0]))

    out, res = _run(nc, x, weight, bias, _trace)
    if not _spot_check(out, x, weight, bias):
        # transient device-side failure (observed ~once per many runs as a
        # single core returning zeros) - run once more
        out, res = _run(nc, x, weight, bias, _trace)
    if _trace:
        return out, res
    return out


# revision 4
# speedup vs baseline: 1.1500x; 1.0347x over previous
"""BinaryLinear Trainium2 kernel.

Computes out = x @ sign(weight).T + bias for x [8192, 4096] f32,
weight [4096, 4096] f32, bias [4096] f32.

Strategy: data-parallel over the token dim across 8 NeuronCores
(1024 tokens per core, weight/bias replicated, no collectives).

Per-core pipeline (no DRAM scratch):
  1. x tiles [128t, 4096i] are cast f32->bf16 during the SWDGE DMA load,
     then one whole-tile XBAR transpose SBUF->SBUF lands each in
     XT [128i, 32k, 1024t] (8 transposes for x, issued first - they gate
     the first matmuls).
  2. weight rows likewise: cast to bf16 SBUF slabs [128o, 4096i]; one
     XBAR transpose per slab half fills WT_n [128i, 16k, 512o]; sign()
     is applied in place on the Scalar engine (scale=1e30 pushes tiny
     values off the LUT's zero neighborhood; sign(0)=0 preserved).
     NOTE: transposes must stay on nc.sync - the scalar HWDGE ring
     corrupts XBAR transposes on HW (passes CoreSim).
  3. TensorE: psum[m] += XT[k,m].T @ WT[n,k], fp32 accumulation in PSUM
     over all 32 k-tiles; two phase-shifted groups of 4 PSUM banks.
  4. DVE adds the (preloaded, partition-broadcast) bias while copying
     PSUM->SBUF; the scalar HWDGE ring stores f32 output tiles.

Scheduling notes (the 3.3x over the first working version):
  - Output stores live on the scalar HWDGE ring, NOT the gpsimd SWDGE
    FIFO: a store queued behind multi-us W cast DMAs stalls DVE ->
    PSUM-bank recycling -> TensorE.
  - All 8 per-block bias tiles are broadcast once at the head.
  - The PE then issues back-to-back ~102ns matmuls (N=512 bf16), which
    is the measured TensorE roofline here.
"""

import numpy as np

import concourse.mybir as mybir
import concourse.tile as tile
from concourse import bacc
from concourse.bass import ts

P = 128
TOKENS, IN_F, OUT_F = 8192, 4096, 4096
N_CORES = 8
N_TILE = 512   # output-feature block (one PSUM bank of f32)

F32 = mybir.dt.float32
BF16 = mybir.dt.bfloat16


def build_nc(t_shard=TOKENS // N_CORES, in_f=IN_F, out_f=OUT_F, repeat=1):
    m_tiles = t_shard // P      # token tiles of 128
    n_tiles = out_f // N_TILE   # output blocks of 512
    ko_tiles = in_f // P        # k tiles of 128
    j_tiles = N_TILE // P       # 128-row slabs per output block

    nc = bacc.Bacc(None, target_bir_lowering=False, debug=False)

    x = nc.dram_tensor("x", [t_shard, in_f], F32, kind="ExternalInput")
    w = nc.dram_tensor("weight", [out_f, in_f], F32, kind="ExternalInput")
    b = nc.dram_tensor("bias", [out_f], F32, kind="ExternalInput")
    out = nc.dram_tensor("out", [t_shard, out_f], F32, kind="ExternalOutput")

    with tile.TileContext(nc) as tc:
        with (
            tc.tile_pool(name="consts", bufs=8) as const_pool,
            tc.tile_pool(name="stage", bufs=6) as stage_pool,
            tc.tile_pool(name="xt", bufs=1) as xt_pool,
            tc.tile_pool(name="wt", bufs=4) as wt_pool,
            tc.tile_pool(name="out_sb", bufs=6) as out_pool,
            tc.tile_pool(name="ps", bufs=8, space="PSUM") as psum_pool,
        ):
          for _rep in range(repeat):
            biases = []
            for n in range(n_tiles):
                bias_rep = const_pool.tile(
                    [P, N_TILE], F32, name=f"bias_rep_{n}", tag="bias"
                )
                nc.gpsimd.dma_start(
                    bias_rep,
                    b[None, ts(n, N_TILE)].broadcast_to([P, N_TILE]),
                )
                biases.append(bias_rep)

            def cast_slab(src_rows):
                """SWDGE cast f32->bf16 of 128 DRAM rows into SBUF."""
                slab = stage_pool.tile([P, in_f], BF16, name="slab", tag="stage")
                nc.gpsimd.dma_start(slab, src_rows)
                return slab

            half_k = max(1, ko_tiles // 2)

            def emit_wt(n):
                """Build signed WT half-tiles [128i, 16k, 512o] for block n."""
                slabs = [
                    cast_slab(w[ts(n * j_tiles + j, P), :])
                    for j in range(j_tiles)
                ]
                halves = []
                for h in range(ko_tiles // half_k):
                    wt_h = wt_pool.tile(
                        [P, half_k, N_TILE], BF16, name=f"wt_{n}_{h}", tag="wt"
                    )
                    for j in range(j_tiles):
                        # NOTE: must stay on nc.sync (see module docstring)
                        nc.sync.dma_start(
                            wt_h[:, :, ts(j, P)],
                            slabs[j][:, ts(h, half_k * P)],
                            transpose=True,
                        )
                    # sign in place; scale pushes tiny magnitudes off the
                    # LUT's zero breakpoint while keeping sign(0) == 0
                    nc.scalar.activation(
                        wt_h, wt_h, mybir.ActivationFunctionType.Sign,
                        scale=1.0e30,
                    )
                    halves.append(wt_h)
                return halves

            # ---- head: x tiles first (each transposed whole so matmuls
            # can start after the first), then W blocks 0 and 1.
            xt_all = xt_pool.tile(
                [P, ko_tiles, t_shard], BF16, name="xt_all", tag="xt"
            )

            def emit_x(m):
                slab = cast_slab(x[ts(m, P), :])
                nc.sync.dma_start(
                    xt_all[:, :, ts(m, P)], slab, transpose=True
                )

            # interleaved head: the first matmul group needs xt m0-3 and
            # wt0 half0 - queue only half the x casts ahead of W block 0
            # so wt0's cast->transpose->sign chain starts ~25us earlier
            for m in range(m_tiles // 2):
                emit_x(m)
            wts = {0: emit_wt(0)}
            for m in range(m_tiles // 2, m_tiles):
                emit_x(m)
            if n_tiles > 1:
                wts[1] = emit_wt(1)

            # ---- main loop over output blocks
            for n in range(n_tiles):
                if n + 2 < n_tiles:
                    wts[n + 2] = emit_wt(n + 2)
                wt_n = wts.pop(n)

                # two phase-shifted groups of 4 PSUM banks: group B's
                # matmuls overlap group A's output copies
                half = max(1, m_tiles // 2)
                for g0 in range(0, m_tiles, half):
                    ms = range(g0, min(g0 + half, m_tiles))
                    psums = {
                        m: psum_pool.tile(
                            [P, N_TILE], F32, name=f"ps_{n}_{m}", tag="ps"
                        )
                        for m in ms
                    }
                    for k in range(ko_tiles):
                        for m in ms:
                            nc.tensor.matmul(
                                psums[m],
                                xt_all[:, k, ts(m, P)],
                                wt_n[k // half_k][:, k % half_k, :],
                                start=(k == 0),
                                stop=(k == ko_tiles - 1),
                            )
                    for m in ms:
                        out_sb = out_pool.tile(
                            [P, N_TILE], F32, name="out_sb", tag="out_sb"
                        )
                        nc.vector.tensor_tensor(
                            out_sb, psums[m], biases[n], mybir.AluOpType.add
                        )
                        nc.scalar.dma_start(
                            out[ts(m, P), ts(n, N_TILE)], out_sb
                        )

    nc.compile()
    return nc


_NC_CACHE = {}


def _get_nc(shape_key):
    if shape_key not in _NC_CACHE:
        _NC_CACHE[shape_key] = build_nc(*shape_key)
    return _NC_CACHE[shape_key]


def _run(nc, x, weight, bias, trace):
    from concourse.bass_utils import run_bass_kernel_spmd

    tokens = x.shape[0]
    t_shard = tokens // N_CORES
    in_maps = [
        {
            "x": x[c * t_shard : (c + 1) * t_shard],
            "weight": weight,
            "bias": bias,
        }
        for c in range(N_CORES)
    ]
    res = run_bass_kernel_spmd(
        nc, in_maps, core_ids=list(range(N_CORES)), trace=trace
    )
    return np.concatenate([r["out"] for r in res.results], axis=0), res


def _spot_check(out, x, weight, bias):
    """Verify 2 sampled output columns against numpy; guards against the
    rare transient where one core's execution returns zeros/garbage."""
    cols = [137, 3972]
    s = np.sign(weight[cols, :].astype(np.float32)).T  # [in_f, 2]
    ref = x.astype(np.float32) @ s + bias[cols][None, :]
    got = out[:, cols]
    denom = np.linalg.norm(ref)
    rel = np.linalg.norm(got - ref) / max(denom, 1e-30)
    return rel < 1e-2


def kernel(x, weight, bias, _trace=False):
    x = np.ascontiguousarray(np.asarray(x, dtype=np.float32))
    weight = np.ascontiguousarray(np.asarray(weight, dtype=np.float32))
    bias = np.ascontiguousarray(np.asarray(bias, dtype=np.float32))

    tokens = x.shape[0]
    t_shard = tokens // N_CORES
    nc = _get_nc((t_shard, x.shape[1], weight.shape[0]))

    out, res = _run(nc, x, weight, bias, _trace)
    if not _spot_check(out, x, weight, bias):
        # transient device-side failure (observed ~once per many runs as a
        # single core returning zeros) - run once more
        out, res = _run(nc, x, weight, bias, _trace)
    if _trace:
        return out, res
    return out
